# revision 8
# baseline (speedup 1.0000x reference)
"""Trainium2 Bass kernel for nn_Attention_25847113187663.

Dense transformer attention block:
    qkv = x @ qkv_w.T ; q,k,v per-head ; attn = softmax(q k^T * scale + bias)
    out = (attn @ v) @ proj_w.T + proj_b
Shapes: x [2, 2048, 512], adj_pos_embed [2, 2047, 2047] (padded to [2048,2048]
additive bias, shared across heads), qkv_w [1536, 512], proj_w [512, 512].

Sharding over 8 cores: batch(2) x query-half(2) x head-half(2).
Each core: 1024 queries, 4 heads, all 2048 keys of one batch.

Per-core design (v2 — scalar-exp-paced flat pipeline):
  - softmax(s+b) = exp(s)*exp(b)/sum: exp(bias) is precomputed on the host,
    so no bias add on device at all. The identity-matmul / fp32 tensor-add of
    v1 are replaced by bf16 multiplies (cheaper, off the critical engines).
  - One flat stream of 64 units (hp, qh, kc): per unit a row-tiled pair of
    K=64 score matmuls (two heads concurrently in the PE array halves),
    one Exp ACT [128,1024] on ScalarE (the pacer), two bf16 multiplies with
    exp(bias) (DVE, alternating one to GpSimd), two attn@v matmuls
    accumulating [d+ones, q] in PSUM. PSUM: 3 rotating 2-bank score tiles +
    2 single-bank attn@v accumulators = exactly 8 banks.
  - qkv projections stream: inputs arrive over 4 DMA queues ordered by need
    time; only head-pair-0's q/k and the first v tiles are computed before
    the unit stream starts; the rest (head-pair 1, remaining v) weave into
    the PE/DVE slack of the first iterations.
  - softmax denominators via a ones-column in v (row 64 of the attn@v
    accumulator); normalization reads sums straight from PSUM, broadcasts
    reciprocals via GpSimd, and the output projection weaves into the tail.
  - ScalarE issues no DMAs and does no copies during the unit stream — it
    only runs Exp; the exp activation table is preloaded with a dummy ACT.
"""

import sys

sys.path.insert(0, "/opt/trn_rl_repo")

import numpy as np

B, N, C, H, D = 2, 2048, 512, 8, 64
SCALE = D**-0.5
Q = 1024  # queries per core
HH = 4  # heads per core
KC = 16  # key chunks of 128
SKEW = 2  # units of lag between exp and attn@v

_prog_cache = {}


def _build_program():
    import concourse.bass as bass  # noqa: F401
    import concourse.tile as tile
    from concourse import bacc, mybir

    fp32 = mybir.dt.float32
    bf16 = mybir.dt.bfloat16
    EXP = mybir.ActivationFunctionType.Exp

    nc = bacc.Bacc("TRN2", target_bir_lowering=False, debug=False, num_devices=8)

    xT_d = nc.dram_tensor("xT", [C, N], bf16, kind="ExternalInput")
    wqT_d = nc.dram_tensor("wqT", [C, HH * D], bf16, kind="ExternalInput")
    wkT_d = nc.dram_tensor("wkT", [C, HH * D], bf16, kind="ExternalInput")
    wvT_d = nc.dram_tensor("wvT", [C, HH * D], bf16, kind="ExternalInput")
    pwT_d = nc.dram_tensor("pwT", [HH * D, C], bf16, kind="ExternalInput")
    ebT_d = nc.dram_tensor("ebT", [N, Q], bf16, kind="ExternalInput")
    out_d = nc.dram_tensor("outp", [Q, C], fp32, kind="ExternalOutput")

    with tile.TileContext(nc) as tc:
        with (
            tc.tile_pool(name="persist", bufs=1) as persist,
            tc.tile_pool(name="at_p", bufs=3) as at_pool,
            tc.tile_pool(name="atm_p", bufs=4) as atm_pool,
            tc.tile_pool(name="nrm_p", bufs=2) as nrm_pool,
            tc.tile_pool(name="out_p", bufs=3) as out_pool,
            tc.tile_pool(name="sp", bufs=3, space="PSUM") as sp_pool,
            tc.tile_pool(name="ot", bufs=1, space="PSUM") as ot_pool,
        ):
            # ---- persistent SBUF ----
            xt = persist.tile([128, 4, N], bf16)  # x[b]^T rolled; part = c-chunk
            wq = persist.tile([128, 4, HH * D], bf16)
            wk = persist.tile([128, 4, HH * D], bf16)
            wv = persist.tile([128, 4, HH * D], bf16)
            pw = persist.tile([128, 2, C], bf16)
            ebt = persist.tile([128, KC, Q], bf16)  # exp(bias)^T chunks
            kT = persist.tile([128, 2, N], bf16)  # [d(2 heads), pair, keys]
            qT = persist.tile([128, 2, Q], bf16)
            v = persist.tile([128, KC, HH, D + 1], bf16)  # ones col at [.., D]
            ao = persist.tile([128, 2, Q], bf16)  # normalized attn-out^T
            warm = persist.tile([128, 16], fp32)  # dummy: preload exp table

            nc.gpsimd.memset(v[:, :, :, D : D + 1], 1.0)
            nc.gpsimd.memset(warm[:, :], 0.0)

            def cast(eng, out, in_):
                if eng is nc.scalar:
                    eng.copy(out, in_)
                else:
                    eng.tensor_copy(out, in_)

            # ---- DMA issue order: 3 queues (SP, Activation, gpsimd SWDGE),
            # ordered by need time. Scalar issues only the early weights +
            # eb0/eb1, all before its exp-pacing duty starts (~8us).
            # sync:   xt cc0, xt cc3, eb 2,4,6,8, pw, eb 10,12,14 (+outs)
            # scalar: wq, wk, wv, eb 0, eb 1, exp-table preload
            # gpsimd: xt cc1, xt cc2, eb 3,5,7,9,11,13,15
            def dma_w(eng, wtile, w_d):
                eng.dma_start(
                    out=wtile[:, :, :],
                    in_=w_d.rearrange("(g p) c -> p g c", p=128),
                )

            def send_eb(eng, kc):
                eng.dma_start(
                    out=ebt[:, kc, :], in_=ebT_d[kc * 128 : (kc + 1) * 128, :]
                )

            nc.sync.dma_start(out=xt[:, 0, :], in_=xT_d[0:128, :])
            nc.sync.dma_start(out=xt[:, 3, :], in_=xT_d[384:512, :])
            dma_w(nc.scalar, wq, wqT_d)
            dma_w(nc.scalar, wk, wkT_d)
            dma_w(nc.scalar, wv, wvT_d)
            send_eb(nc.scalar, 0)
            send_eb(nc.scalar, 1)
            # preload the exp activation table while scalar is otherwise idle
            nc.scalar.activation(warm[:, :], warm[:, :], EXP)
            nc.gpsimd.dma_start(out=xt[:, 1, :], in_=xT_d[128:256, :])
            nc.gpsimd.dma_start(out=xt[:, 2, :], in_=xT_d[256:384, :])
            for kc in (2, 4, 6, 8):
                send_eb(nc.sync, kc)
            nc.sync.dma_start(
                out=pw[:, :, :],
                in_=pwT_d.rearrange("(g p) c -> p g c", p=128),
            )
            for kc in (10, 12, 14):
                send_eb(nc.sync, kc)
            for kc in (3, 5, 7, 9, 11, 13, 15):
                send_eb(nc.gpsimd, kc)

            # ---- phase A building blocks ----
            def proj_qk(dst, wsrc, dc, nsl, cast_eng):
                """dst[:, dc, nsl] = (w-chunk)^T @ xT over two 512-col halves."""
                sp = sp_pool.tile([128, 2, 512], fp32, tag="sp", name="spa")
                n0 = nsl.start
                for j in range(2):
                    for cc in range(4):
                        nc.tensor.matmul(
                            sp[:, j, :],
                            lhsT=wsrc[:, cc, dc * 128 : (dc + 1) * 128],
                            rhs=xt[:, cc, n0 + j * 512 : n0 + (j + 1) * 512],
                            start=(cc == 0),
                            stop=(cc == 3),
                        )
                cast(cast_eng, dst[:, dc, n0 : n0 + 1024], sp[:, :, :])

            def proj_v(t0, cast_eng):
                """v tiles t0, t0+1 (128 tokens each, all 4 heads)."""
                sp = sp_pool.tile([128, 2, 512], fp32, tag="sp", name="spv")
                for j in range(2):
                    for cc in range(4):
                        nc.tensor.matmul(
                            sp[:, j, 0 : HH * D],
                            lhsT=xt[:, cc, (t0 + j) * 128 : (t0 + j + 1) * 128],
                            rhs=wv[:, cc, :],
                            start=(cc == 0),
                            stop=(cc == 3),
                        )
                cast(
                    cast_eng,
                    v[:, t0 : t0 + 2, :, 0:D],
                    sp[:, :, 0 : HH * D].rearrange("p t (h d) -> p t h d", h=HH),
                )

            # ---- critical prefix: head-pair 0 q/k, first v tiles ----
            proj_qk(qT, wq, 0, slice(0, Q), nc.vector)
            proj_qk(kT, wk, 0, slice(0, 1024), nc.vector)
            proj_qk(kT, wk, 0, slice(1024, 2048), nc.vector)
            proj_v(0, nc.vector)
            proj_v(2, nc.scalar)
            proj_v(4, nc.scalar)
            proj_v(6, nc.scalar)

            # remaining phase-A work, woven into early units (emit at unit g)
            weave = {
                2: lambda: proj_v(8, nc.vector),
                4: lambda: proj_v(10, nc.vector),
                6: lambda: proj_qk(qT, wq, 1, slice(0, Q), nc.vector),
                8: lambda: proj_qk(kT, wk, 1, slice(0, 1024), nc.vector),
                10: lambda: proj_qk(kT, wk, 1, slice(1024, 2048), nc.vector),
                12: lambda: proj_v(12, nc.vector),
                14: lambda: proj_v(14, nc.vector),
            }

            # ---- flat unit stream ----
            iters = [(0, 0), (0, 1), (1, 0), (1, 1)]  # (hp, qh)
            oT = {}  # iteration -> [oT_hi0, oT_hi1]
            pend = []  # (it, kc, atm) awaiting attn@v
            out_done = [0]

            def emit_av(it, kc, atm):
                hp, _ = iters[it]
                for hi in range(2):
                    nc.tensor.matmul(
                        oT[it][hi][0 : D + 1, :],
                        lhsT=v[:, kc, hp * 2 + hi, :],
                        rhs=atm[:, hi, :],
                        start=(kc == 0),
                        stop=(kc == KC - 1),
                    )

            def emit_norm(it):
                hp, qh = iters[it]
                qsl = slice(qh * 512, (qh + 1) * 512)
                for hi in range(2):
                    srow = nrm_pool.tile([1, 512], fp32, tag="srow", name="srow")
                    nc.vector.tensor_copy(srow[:, :], oT[it][hi][D : D + 1, :])
                    rbc = nrm_pool.tile([64, 512], fp32, tag="rbc", name="rbc")
                    nc.gpsimd.partition_broadcast(rbc[:, :], srow[:, :])
                    nc.vector.reciprocal_approx_fast(rbc[:, :], rbc[:, :])
                    nc.vector.tensor_mul(
                        ao[hi * 64 : (hi + 1) * 64, hp, qsl],
                        oT[it][hi][0:D, :],
                        rbc[:, :],
                    )

            def emit_out(qc, ev_eng, dma_eng):
                po = sp_pool.tile([128, 2, 512], fp32, tag="sp", name="po")
                for cc in range(2):
                    nc.tensor.matmul(
                        po[:, 0, :],
                        lhsT=ao[:, cc, qc * 128 : (qc + 1) * 128],
                        rhs=pw[:, cc, :],
                        start=(cc == 0),
                        stop=(cc == 1),
                    )
                ot = out_pool.tile([128, C], fp32, tag="ot", name="ot")
                cast(ev_eng, ot[:, :], po[:, 0, :])
                dma_eng.dma_start(
                    out=out_d[qc * 128 : (qc + 1) * 128, :], in_=ot[:, :]
                )
                out_done[0] += 1

            for g in range(64):
                it, kc = g // KC, g % KC
                hp, qh = iters[it]
                qsl = slice(qh * 512, (qh + 1) * 512)
                kcs = slice(kc * 128, (kc + 1) * 128)
                if kc == 0:
                    oT[it] = [
                        ot_pool.tile(
                            [D + 1, 512], fp32, tag=f"o{hi}", name=f"oT{it}{hi}"
                        )
                        for hi in range(2)
                    ]
                sp = sp_pool.tile([128, 2, 512], fp32, tag="sp", name="sps")
                for hi in range(2):
                    lo = hi * 64
                    nc.tensor.matmul(
                        sp[:, hi, :],
                        lhsT=kT[lo : lo + 64, hp, kcs],
                        rhs=qT[lo : lo + 64, hp, qsl],
                        tile_position=(lo, 0),
                        start=True,
                        stop=True,
                    )
                at = at_pool.tile([128, 2, 512], bf16, tag="at", name="at")
                nc.scalar.activation(at[:, :, :], sp[:, :, :], EXP)
                atm = atm_pool.tile([128, 2, 512], bf16, tag="atm", name="atm")
                nc.vector.tensor_mul(atm[:, 0, :], at[:, 0, :], ebt[:, kc, qsl])
                mul1_eng = nc.gpsimd if (g % 2 == 1) else nc.vector
                mul1_eng.tensor_mul(atm[:, 1, :], at[:, 1, :], ebt[:, kc, qsl])

                pend.append((it, kc, atm))
                if len(pend) > SKEW:
                    emit_av(*pend.pop(0))
                    if kc == SKEW + 1 and it > 0:
                        emit_norm(it - 1)  # prev iteration fully accumulated
                if g in weave:
                    weave[g]()
                # output projection for query-half 0 after its last norm
                # (it2 = (hp1, qh0) norm lands at g=51)
                if g in (53, 55, 57, 59):
                    emit_out((g - 53) // 2, nc.vector, nc.sync)

            while pend:
                emit_av(*pend.pop(0))
            emit_norm(3)
            for qc in range(4, 8):
                emit_out(qc, nc.scalar, nc.sync if qc % 2 == 0 else nc.gpsimd)

    nc.finalize()
    return nc


def _get_program():
    if "nc" not in _prog_cache:
        _prog_cache["nc"] = _build_program()
    return _prog_cache["nc"]


def _shard_inputs(x, adj_pos_embed, qkv_w, proj_w):
    """Build the 8 per-core input maps (host-side layout prep)."""
    import ml_dtypes

    x = np.asarray(x, dtype=np.float32)
    adj = np.asarray(adj_pos_embed, dtype=np.float32)
    qkv_w = np.asarray(qkv_w, dtype=np.float32)
    proj_w = np.asarray(proj_w, dtype=np.float32)

    # exp of padded bias, transposed: ebfull[b, k, q] = exp(pad(adj[b])[q, k])
    ebfull = np.ones((B, N, N), dtype=np.float32)
    for b in range(B):
        ebfull[b, : N - 1, : N - 1] = np.exp(adj[b].T)

    in_maps = []
    for core in range(8):
        b = core // 4
        qh = (core // 2) % 2
        hh = core % 2
        qoff = qh * Q
        # roll tokens so this core's queries are the first Q columns of xT;
        # bias rows are rolled identically so key indexing stays consistent
        xT = np.ascontiguousarray(np.roll(x[b], -qoff, axis=0).T).astype(
            ml_dtypes.bfloat16
        )
        ebT = np.ascontiguousarray(
            np.roll(ebfull[b, :, qoff : qoff + Q], -qoff, axis=0)
        ).astype(ml_dtypes.bfloat16)
        r0 = hh * (HH * D)
        wq = qkv_w[0 * C + r0 : 0 * C + r0 + HH * D, :]  # [256, 512]
        wk = qkv_w[1 * C + r0 : 1 * C + r0 + HH * D, :]
        wv = qkv_w[2 * C + r0 : 2 * C + r0 + HH * D, :]
        wqT = (np.ascontiguousarray(wq.T) * np.float32(SCALE)).astype(
            ml_dtypes.bfloat16
        )
        wkT = np.ascontiguousarray(wk.T).astype(ml_dtypes.bfloat16)
        wvT = np.ascontiguousarray(wv.T).astype(ml_dtypes.bfloat16)
        pwT = np.ascontiguousarray(proj_w[:, r0 : r0 + HH * D].T).astype(
            ml_dtypes.bfloat16
        )
        in_maps.append(
            {"xT": xT, "wqT": wqT, "wkT": wkT, "wvT": wvT, "pwT": pwT, "ebT": ebT}
        )
    return in_maps


def kernel(x, adj_pos_embed, qkv_w, proj_w, proj_b, _trace=False):
    from concourse.bass_utils import run_bass_kernel_spmd

    nc = _get_program()
    in_maps = _shard_inputs(x, adj_pos_embed, qkv_w, proj_w)
    res = run_bass_kernel_spmd(nc, in_maps, core_ids=list(range(8)), trace=_trace)
    out = np.zeros((B, N, C), dtype=np.float32)
    for core in range(8):
        b = core // 4
        qh = (core // 2) % 2
        out[b, qh * Q : (qh + 1) * Q, :] += res.results[core]["outp"]
    out += np.asarray(proj_b, dtype=np.float32)[None, None, :]
    if _trace:
        kernel.last_exec_time_ns = res.exec_time_ns
        kernel.last_results = res
    return out


# revision 16
# speedup vs baseline: 1.6928x; 1.6928x over previous
"""Trainium2 Bass kernel for nn_Attention_25847113187663.

Dense transformer attention block:
    qkv = x @ qkv_w.T ; q,k,v per-head ; attn = softmax(q k^T * scale + bias)
    out = (attn @ v) @ proj_w.T + proj_b
Shapes: x [2, 2048, 512], adj_pos_embed [2, 2047, 2047] (padded to [2048,2048]
additive bias, shared across heads), qkv_w [1536, 512], proj_w [512, 512].

Sharding over 8 cores: batch(2) x query-half(2) x head-half(2).
Each core: 1024 queries, 4 heads, all 2048 keys of one batch.

Per-core design (v3 — scalar-exp-paced flat pipeline):
  - softmax(s+b) = exp(s)*exp(b)/sum: exp(bias) precomputed on the host, so
    there is no bias add on device — just bf16 multiplies after the exp.
  - One flat stream of 64 units (hp, qh, kc): per unit a row-tiled pair of
    K=64 score matmuls (two heads concurrently in the PE array halves), one
    Exp ACT [128,1024] on ScalarE (the pacer), two bf16 exp(bias) multiplies
    (even units: both on DVE; odd units: one on DVE one on GpSimd), two
    attn@v matmuls accumulating [d+ones, q] in PSUM. PSUM: 3 rotating
    2-bank score tiles + 2 single-bank attn@v accumulators = 8 banks.
  - GpSimd runs ONLY tensor_tensor (plus memset/DMA-gen, which need no DSP
    library): a library swap costs ~6us, so the softmax-denominator
    broadcast is a K=2 matmul on the PE (selector matrix x sums row) into a
    rotating score-tile slot, then one DVE reciprocal covers both heads.
  - DMA: gpsimd's SWDGE queue is ~30GB/s vs ~110GB/s for the sync/scalar
    HWDGE queues, so xT/weights ride only the fast queues, ordered by need
    time; gpsimd carries late-needed exp(bias) chunks + proj weights. All
    scalar-queue issues happen in the prologue (issue cost would otherwise
    stall the exp pacing).
  - qkv projections: only q/k of head-pair 0 and the first v tile precede
    the unit stream; everything else weaves into early-unit PE/DVE slack.
  - Output projection weaves into the last iteration; outputs leave as bf16
    (host accumulates in fp32 and adds proj_b).
"""

import sys

sys.path.insert(0, "/opt/trn_rl_repo")

import numpy as np

B, N, C, H, D = 2, 2048, 512, 8, 64
SCALE = D**-0.5
Q = 1024  # queries per core
HH = 4  # heads per core
KC = 16  # key chunks of 128
SKEW = 2  # units of lag between exp and attn@v

_prog_cache = {}


def _build_program():
    import concourse.bass as bass  # noqa: F401
    import concourse.tile as tile
    from concourse import bacc, mybir

    fp32 = mybir.dt.float32
    bf16 = mybir.dt.bfloat16
    EXP = mybir.ActivationFunctionType.Exp

    nc = bacc.Bacc("TRN2", target_bir_lowering=False, debug=False, num_devices=8)

    xT_d = nc.dram_tensor("xT", [C, N], bf16, kind="ExternalInput")
    wqT_d = nc.dram_tensor("wqT", [C, HH * D], bf16, kind="ExternalInput")
    wkT_d = nc.dram_tensor("wkT", [C, HH * D], bf16, kind="ExternalInput")
    wvT_d = nc.dram_tensor("wvT", [C, HH * D], bf16, kind="ExternalInput")
    pwT_d = nc.dram_tensor("pwT", [HH * D, C], bf16, kind="ExternalInput")
    ebT_d = nc.dram_tensor("ebT", [N, Q], bf16, kind="ExternalInput")
    bsel_d = nc.dram_tensor("bsel", [2, 128], bf16, kind="ExternalInput")
    out_d = nc.dram_tensor("outp", [Q, C], bf16, kind="ExternalOutput")

    with tile.TileContext(nc) as tc:
        with (
            tc.tile_pool(name="persist", bufs=1) as persist,
            tc.tile_pool(name="at_p", bufs=4) as at_pool,
            tc.tile_pool(name="atm_p", bufs=4) as atm_pool,
            tc.tile_pool(name="nrm_p", bufs=2) as nrm_pool,
            tc.tile_pool(name="out_p", bufs=3) as out_pool,
            tc.tile_pool(name="sp", bufs=3, space="PSUM") as sp_pool,
            tc.tile_pool(name="ot", bufs=1, space="PSUM") as ot_pool,
        ):
            # ---- persistent SBUF ----
            xt = persist.tile([128, 4, N], bf16)  # x[b]^T rolled; part = c-chunk
            wq = persist.tile([128, 4, HH * D], bf16)
            wk = persist.tile([128, 4, HH * D], bf16)
            wv = persist.tile([128, 4, HH * D], bf16)
            pw = persist.tile([128, 2, C], bf16)
            ebt = persist.tile([128, KC, Q], bf16)  # exp(bias)^T chunks
            kT = persist.tile([128, 2, N], bf16)  # [d(2 heads), pair, keys]
            qT = persist.tile([128, 2, Q], bf16)
            v = persist.tile([128, KC, HH, D + 1], bf16)  # ones col at [.., D]
            ao = persist.tile([128, 2, Q], bf16)  # normalized attn-out^T
            bsel = persist.tile([1, 2, 128], bf16)  # hi-broadcast selector rows
            warmb = persist.tile([128, 32], bf16)  # engine warm-up scratch
            wo1 = persist.tile([128, 32], bf16)
            wo2 = persist.tile([128, 32], bf16)

            nc.vector.memset(v[:, :, :, D : D + 1], 1.0)
            nc.vector.memset(warmb[:, :], 0.0)

            # ---- DMA issues, 3 queues, ordered by need time ----
            # sync (SP):     wk, xt0, xt2, eb 0,2,4,6,8,10,15  (+outs later)
            # scalar (ACT):  wq, xt1, xt3, wv, eb 1,3,5,7,9,11 (all prologue)
            # gpsimd (SWDGE ~30GB/s): eb 12,13,14, pw (late-needed only)
            def dma_w(eng, wtile, w_d):
                eng.dma_start(
                    out=wtile[:, :, :],
                    in_=w_d.rearrange("(g p) c -> p g c", p=128),
                )

            def send_xt(eng, cc):
                eng.dma_start(
                    out=xt[:, cc, :], in_=xT_d[cc * 128 : (cc + 1) * 128, :]
                )

            def send_eb(eng, kc):
                eng.dma_start(
                    out=ebt[:, kc, :], in_=ebT_d[kc * 128 : (kc + 1) * 128, :]
                )

            nc.sync.dma_start(out=bsel[0:1, :, :], in_=bsel_d[:, :])
            dma_w(nc.sync, wk, wkT_d)
            send_xt(nc.sync, 0)
            send_xt(nc.sync, 2)
            dma_w(nc.scalar, wq, wqT_d)
            send_xt(nc.scalar, 1)
            send_xt(nc.scalar, 3)
            dma_w(nc.scalar, wv, wvT_d)
            for kc in (0, 2, 4, 6, 8, 10):
                send_eb(nc.sync, kc)
            send_eb(nc.sync, 15)
            for kc in (1, 3, 5, 7, 9, 11):
                send_eb(nc.scalar, kc)
            for kc in (12, 13, 14):
                send_eb(nc.gpsimd, kc)
            dma_w(nc.gpsimd, pw, pwT_d)
            # warm-ups: preload exp table (scalar) and the TT DSP library
            # (gpsimd) while the input DMAs are in flight
            nc.scalar.activation(wo1[:, :], warmb[:, :], EXP)
            nc.gpsimd.tensor_mul(wo2[:, :], warmb[:, :], warmb[:, :])

            # ---- phase A building blocks ----
            def proj_qk(dst, wsrc, dc, nsl):
                """dst[:, dc, nsl] = (w-chunk)^T @ xT over two 512-col halves."""
                sp = sp_pool.tile([128, 2, 512], fp32, tag="sp", name="spa")
                n0 = nsl.start
                for j in range(2):
                    for cc in range(4):
                        nc.tensor.matmul(
                            sp[:, j, :],
                            lhsT=wsrc[:, cc, dc * 128 : (dc + 1) * 128],
                            rhs=xt[:, cc, n0 + j * 512 : n0 + (j + 1) * 512],
                            start=(cc == 0),
                            stop=(cc == 3),
                        )
                nc.vector.tensor_copy(dst[:, dc, n0 : n0 + 1024], sp[:, :, :])

            def proj_v(t0):
                """v tiles t0, t0+1 (128 tokens each, all 4 heads)."""
                sp = sp_pool.tile([128, 2, 512], fp32, tag="sp", name="spv")
                for j in range(2):
                    for cc in range(4):
                        nc.tensor.matmul(
                            sp[:, j, 0 : HH * D],
                            lhsT=xt[:, cc, (t0 + j) * 128 : (t0 + j + 1) * 128],
                            rhs=wv[:, cc, :],
                            start=(cc == 0),
                            stop=(cc == 3),
                        )
                nc.vector.tensor_copy(
                    v[:, t0 : t0 + 2, :, 0:D],
                    sp[:, :, 0 : HH * D].rearrange("p t (h d) -> p t h d", h=HH),
                )

            # ---- minimal critical prefix ----
            proj_qk(qT, wq, 0, slice(0, Q))
            proj_qk(kT, wk, 0, slice(0, 1024))  # kc 0-7
            proj_v(0)

            # remaining phase-A work, woven into early units (emit at unit g)
            weave = {
                0: lambda: proj_qk(kT, wk, 0, slice(1024, 2048)),  # kc 8-15
                1: lambda: proj_v(2),
                2: lambda: proj_v(4),
                3: lambda: proj_v(6),
                4: lambda: proj_v(8),
                5: lambda: proj_qk(qT, wq, 1, slice(0, Q)),
                6: lambda: proj_v(10),
                8: lambda: proj_qk(kT, wk, 1, slice(0, 1024)),
                10: lambda: proj_qk(kT, wk, 1, slice(1024, 2048)),
                12: lambda: proj_v(12),
                14: lambda: proj_v(14),
            }

            # ---- flat unit stream ----
            iters = [(0, 0), (0, 1), (1, 0), (1, 1)]  # (hp, qh)
            oT = {}  # iteration -> [oT_hi0, oT_hi1]
            pend = []  # (it, kc, atm) awaiting attn@v

            def emit_av(it, kc, atm):
                hp, _ = iters[it]
                for hi in range(2):
                    nc.tensor.matmul(
                        oT[it][hi][0 : D + 1, :],
                        lhsT=v[:, kc, hp * 2 + hi, :],
                        rhs=atm[:, hi, :],
                        start=(kc == 0),
                        stop=(kc == KC - 1),
                    )
                if kc == KC - 1:
                    emit_norm(it)

            def emit_norm(it):
                hp, qh = iters[it]
                qsl = slice(qh * 512, (qh + 1) * 512)
                srow = nrm_pool.tile([1, 2, 512], bf16, tag="srow", name="srow")
                for hi in range(2):
                    nc.vector.tensor_copy(
                        srow[0:1, hi, :], oT[it][hi][D : D + 1, :]
                    )
                # broadcast sums to the hi-matched partition halves via two
                # accumulating K=1 matmuls with disjoint selector rows
                # (GpSimd partition_broadcast would thrash DSP libraries)
                rbp = sp_pool.tile([128, 2, 512], fp32, tag="sp", name="rbp")
                for hi in range(2):
                    nc.tensor.matmul(
                        rbp[:, 0, :],
                        lhsT=bsel[0:1, hi, :],
                        rhs=srow[0:1, hi, :],
                        start=(hi == 0),
                        stop=(hi == 1),
                    )
                rbc = nrm_pool.tile([128, 512], fp32, tag="rbc", name="rbc")
                nc.vector.reciprocal_approx_fast(rbc[:, :], rbp[:, 0, :])
                for hi in range(2):
                    nc.vector.tensor_mul(
                        ao[hi * 64 : (hi + 1) * 64, hp, qsl],
                        oT[it][hi][0:D, :],
                        rbc[hi * 64 : (hi + 1) * 64, :],
                    )

            def emit_out(qc, ev_eng, dma_eng):
                po = sp_pool.tile([128, 2, 512], fp32, tag="sp", name="po")
                for cc in range(2):
                    nc.tensor.matmul(
                        po[:, 0, :],
                        lhsT=ao[:, cc, qc * 128 : (qc + 1) * 128],
                        rhs=pw[:, cc, :],
                        start=(cc == 0),
                        stop=(cc == 1),
                    )
                ot = out_pool.tile([128, C], bf16, tag="ot", name="ot")
                if ev_eng is nc.scalar:
                    ev_eng.copy(ot[:, :], po[:, 0, :])
                else:
                    ev_eng.tensor_copy(ot[:, :], po[:, 0, :])
                dma_eng.dma_start(
                    out=out_d[qc * 128 : (qc + 1) * 128, :], in_=ot[:, :]
                )

            for g in range(64):
                it, kc = g // KC, g % KC
                hp, qh = iters[it]
                qsl = slice(qh * 512, (qh + 1) * 512)
                kcs = slice(kc * 128, (kc + 1) * 128)
                if kc == 0:
                    oT[it] = [
                        ot_pool.tile(
                            [D + 1, 512], fp32, tag=f"o{hi}", name=f"oT{it}{hi}"
                        )
                        for hi in range(2)
                    ]
                sp = sp_pool.tile([128, 2, 512], fp32, tag="sp", name="sps")
                for hi in range(2):
                    lo = hi * 64
                    nc.tensor.matmul(
                        sp[:, hi, :],
                        lhsT=kT[lo : lo + 64, hp, kcs],
                        rhs=qT[lo : lo + 64, hp, qsl],
                        tile_position=(lo, 0),
                        start=True,
                        stop=True,
                    )
                at = at_pool.tile([128, 2, 512], bf16, tag="at", name="at")
                nc.scalar.activation(at[:, :, :], sp[:, :, :], EXP)
                atm = atm_pool.tile([128, 2, 512], bf16, tag="atm", name="atm")
                nc.vector.tensor_mul(atm[:, 0, :], at[:, 0, :], ebt[:, kc, qsl])
                mul1_eng = nc.gpsimd if (g % 2 == 1) else nc.vector
                mul1_eng.tensor_mul(atm[:, 1, :], at[:, 1, :], ebt[:, kc, qsl])

                pend.append((it, kc, atm))
                if len(pend) > SKEW:
                    emit_av(*pend.pop(0))
                if g in weave:
                    weave[g]()
                # output projection for query-half 0 after norm(it2) (g=49)
                if g in (53, 55, 57, 59):
                    emit_out((g - 53) // 2, nc.vector, nc.sync)

            while pend:
                emit_av(*pend.pop(0))
            for qc in range(4, 8):
                emit_out(qc, nc.scalar, nc.sync if qc % 2 == 0 else nc.scalar)

    nc.finalize()
    return nc


def _get_program():
    if "nc" not in _prog_cache:
        _prog_cache["nc"] = _build_program()
    return _prog_cache["nc"]


def _shard_inputs(x, adj_pos_embed, qkv_w, proj_w):
    """Build the 8 per-core input maps (host-side layout prep)."""
    import ml_dtypes

    x = np.asarray(x, dtype=np.float32)
    adj = np.asarray(adj_pos_embed, dtype=np.float32)
    qkv_w = np.asarray(qkv_w, dtype=np.float32)
    proj_w = np.asarray(proj_w, dtype=np.float32)

    # exp of padded bias, transposed: ebfull[b, k, q] = exp(pad(adj[b])[q, k])
    ebfull = np.ones((B, N, N), dtype=np.float32)
    for b in range(B):
        ebfull[b, : N - 1, : N - 1] = np.exp(adj[b].T)

    in_maps = []
    for core in range(8):
        b = core // 4
        qh = (core // 2) % 2
        hh = core % 2
        qoff = qh * Q
        # roll tokens so this core's queries are the first Q columns of xT;
        # bias rows are rolled identically so key indexing stays consistent
        xT = np.ascontiguousarray(np.roll(x[b], -qoff, axis=0).T).astype(
            ml_dtypes.bfloat16
        )
        ebT = np.ascontiguousarray(
            np.roll(ebfull[b, :, qoff : qoff + Q], -qoff, axis=0)
        ).astype(ml_dtypes.bfloat16)
        r0 = hh * (HH * D)
        wq = qkv_w[0 * C + r0 : 0 * C + r0 + HH * D, :]  # [256, 512]
        wk = qkv_w[1 * C + r0 : 1 * C + r0 + HH * D, :]
        wv = qkv_w[2 * C + r0 : 2 * C + r0 + HH * D, :]
        wqT = (np.ascontiguousarray(wq.T) * np.float32(SCALE)).astype(
            ml_dtypes.bfloat16
        )
        wkT = np.ascontiguousarray(wk.T).astype(ml_dtypes.bfloat16)
        wvT = np.ascontiguousarray(wv.T).astype(ml_dtypes.bfloat16)
        pwT = np.ascontiguousarray(proj_w[:, r0 : r0 + HH * D].T).astype(
            ml_dtypes.bfloat16
        )
        bsel = np.zeros((2, 128), dtype=ml_dtypes.bfloat16)
        bsel[0, 0:64] = 1.0
        bsel[1, 64:128] = 1.0
        in_maps.append(
            {
                "xT": xT,
                "wqT": wqT,
                "wkT": wkT,
                "wvT": wvT,
                "pwT": pwT,
                "ebT": ebT,
                "bsel": bsel,
            }
        )
    return in_maps


def kernel(x, adj_pos_embed, qkv_w, proj_w, proj_b, _trace=False):
    from concourse.bass_utils import run_bass_kernel_spmd

    nc = _get_program()
    in_maps = _shard_inputs(x, adj_pos_embed, qkv_w, proj_w)
    res = run_bass_kernel_spmd(nc, in_maps, core_ids=list(range(8)), trace=_trace)
    out = np.zeros((B, N, C), dtype=np.float32)
    for core in range(8):
        b = core // 4
        qh = (core // 2) % 2
        out[b, qh * Q : (qh + 1) * Q, :] += np.asarray(
            res.results[core]["outp"], dtype=np.float32
        )
    out += np.asarray(proj_b, dtype=np.float32)[None, None, :]
    if _trace:
        kernel.last_exec_time_ns = res.exec_time_ns
        kernel.last_results = res
    return out
